# revision 12
# baseline (speedup 1.0000x reference)
"""AliasFreeConv Trainium2 kernel (bf16 pipeline, DRAM corner turns).

Data-parallel over batch: 8 samples -> 8 NeuronCores, no collectives.
Per core, all heavy math in bf16 (PE 1 cycle/row, rel-err ~3e-3 vs the
2e-2 gate):

  phase 0: style matvec (PE, fp32) -> s[ci]; x scaled by s in-place;
           demod g[co] = wscale*rsqrt(wscale^2 * s^2.Q + eps)/(1+eps)
           via host-precomputed Q[ci,co] = sum_k conv_w^2.
  phase 1: per w-column-pair j: 3x3 VALID conv as 36 shifted bf16
           matmuls -> psum; ct = psum*g + bias (DVE); H-up via Ulo/Uhi
           [128x128] matmuls (junk rows killed by zero filter rows);
           u1[w] = [128 hu, 512 co] bf16 -> DRAM (contiguous).
  phase 2: corner turn #1: chunked strided reads u1 -> ubig[w-part,
           hu x co] (hu<64 at partitions 0.., hu>=64 at 64..);
           per (m=co-half, hu): W-up matmul (K=62) -> Prelu (alternating
           ACT/DVE, sqrt2 folded into W-down matrix) -> W-down -> slab
           -> d2[hu] DRAM (corner turn #2).
  phase 3: chunked strided reads d2 -> et[hu-part, wd x co]; H-down
           matmuls -> out stores. m=0 H-down interleaved into m=1's
           W-stage to hide the turn-2 read latency.

Corner turns bounce through DRAM because SBUF->SBUF scatters into few
partitions run at ~20 GB/s (measured), while DRAM strided reads with
>=512B runs sustain ~190 GB/s and contiguous writes ~390 GB/s.
"""
import math
import os
import numpy as np
from contextlib import ExitStack

import concourse.bass as bass
import concourse.bacc as bacc
import concourse.tile as tile
from concourse import mybir
from concourse.bass_utils import run_bass_kernel_spmd

F32 = mybir.dt.float32
BF16 = mybir.dt.bfloat16
AF = mybir.ActivationFunctionType
ALU = mybir.AluOpType

B, CI, CO, H, W = 8, 512, 512, 64, 64
KS, TAPS, UP = 3, 12, 2
HO = H - 2                      # 62 valid conv outputs per axis
STYLE = 512
XPAD = 66 * 64                  # x tile free size: 64 cols x 66 rows
NB = HO // 2                    # 31 w-column-pair blocks
K9 = KS * KS
LIN_SCALE = 1.0 / math.sqrt(STYLE)
WSCALE = 1.0 / math.sqrt(CI * KS * KS)
EPS = 1e-8
SQRT2 = math.sqrt(2.0)
CQ = 256                        # co half for phases 2/3

_CACHE: dict = {}


def _build_nc(reps=1):
    nc = bacc.Bacc()

    xt_d = nc.declare_dram_parameter("xt", [4, 128, XPAD], BF16, isOutput=False)
    stylec_d = nc.declare_dram_parameter("stylec", [4, 128, 1], F32, isOutput=False)
    modw_d = nc.declare_dram_parameter("modw", [4, 128, 512], F32, isOutput=False)
    modb_d = nc.declare_dram_parameter("modb", [128, 4], F32, isOutput=False)
    qh_d = nc.declare_dram_parameter("qh", [4, 128, CO], F32, isOutput=False)
    cw_d = nc.declare_dram_parameter("cw", [4, 128, K9 * CO], BF16, isOutput=False)
    actb_d = nc.declare_dram_parameter("actb", [1, CO], F32, isOutput=False)
    ulo_d = nc.declare_dram_parameter("ulo", [128, 128], BF16, isOutput=False)
    uhi_d = nc.declare_dram_parameter("uhi", [128, 128], BF16, isOutput=False)
    uw_d = nc.declare_dram_parameter("uw", [128, 128], BF16, isOutput=False)
    dw_d = nc.declare_dram_parameter("dw", [128, 64], BF16, isOutput=False)
    dh_d = nc.declare_dram_parameter("dh", [128, 64], BF16, isOutput=False)
    out_d = nc.declare_dram_parameter("o", [64, 64, CO], F32, isOutput=True)

    # u1 split in two tensors so the w<48 corner-turn-1 reads depend
    # only on the first 24 j-blocks (DRAM deps are tensor-granular)
    u1a_d = nc.dram_tensor("u1scratcha", [48, 128, CO], BF16)  # w 0..47
    u1b_d = nc.dram_tensor("u1scratchb", [HO - 48, 128, CO], BF16)
    d2_d = nc.dram_tensor("d2scratch", [2, 64, 128, CQ], BF16)  # [m, wd, hu, co]

    with ExitStack() as ctx:
        tc = ctx.enter_context(tile.TileContext(nc))
        if reps > 1:
            ctx.enter_context(tc.For_i(0, reps, 1))
        pp = ctx.enter_context(tc.tile_pool(name="persist", bufs=1))

        ulo_t = pp.tile([128, 128], BF16)
        uhi_t = pp.tile([128, 128], BF16)
        uw_t = pp.tile([128, 128], BF16)
        dw_t = pp.tile([128, 64], BF16)
        dh_t = pp.tile([128, 64], BF16)
        g_b = pp.tile([128, CO], F32)
        bias_b = pp.tile([128, CO], F32)
        s_col = pp.tile([128, 4], F32)
        ubig = pp.tile([128, 64 * CO], BF16)     # [w(+64 for hu>=64), hu%64 x co]

        with tc.tile_pool(name="xw", bufs=1) as xw, \
             tc.tile_pool(name="pre", bufs=1) as pre, \
             tc.tile_pool(name="preps", bufs=1, space="PSUM") as preps, \
             tc.tile_pool(name="cpool", bufs=3) as cpool, \
             tc.tile_pool(name="upool", bufs=3) as upool, \
             tc.tile_pool(name="ps1", bufs=2, space="PSUM") as ps1, \
             tc.tile_pool(name="ps2", bufs=2, space="PSUM") as ps2:

            xts = [xw.tile([128, XPAD], BF16, tag=f"x{t}", name=f"x{t}") for t in range(4)]
            cws = [xw.tile([128, K9 * CO], BF16, tag=f"w{t}", name=f"w{t}") for t in range(4)]
            modw_t = [pre.tile([128, 512], F32, tag=f"mw{t}", name=f"mw{t}") for t in range(4)]
            styl_t = [pre.tile([128, 1], F32, tag=f"st{t}", name=f"st{t}") for t in range(4)]
            qh_t = [pre.tile([128, CO], F32, tag=f"qh{t}", name=f"qh{t}") for t in range(4)]
            modb_t = pre.tile([128, 4], F32)
            actb_t = pre.tile([1, CO], F32)
            ones_row = pre.tile([1, 128], F32)
            s2_t = pre.tile([128, 4], F32)
            sd_row = pre.tile([1, CO], F32)
            grow = pre.tile([1, CO], F32)
            eps_t = pre.tile([1, 1], F32)

            # q1 (sync): small prologue tensors first, then conv weights.
            for t in range(4):
                nc.sync.dma_start(out=modw_t[t], in_=modw_d[t])
                nc.sync.dma_start(out=styl_t[t], in_=stylec_d[t])
            nc.sync.dma_start(out=modb_t, in_=modb_d[:, :])
            nc.sync.dma_start(out=ulo_t, in_=ulo_d[:, :])
            nc.sync.dma_start(out=uhi_t, in_=uhi_d[:, :])
            nc.sync.dma_start(out=uw_t, in_=uw_d[:, :])
            nc.sync.dma_start(out=dw_t, in_=dw_d[:, :])
            nc.sync.dma_start(out=dh_t, in_=dh_d[:, :])
            for t in range(4):
                nc.sync.dma_start(out=qh_t[t], in_=qh_d[t])
            nc.sync.dma_start(out=actb_t, in_=actb_d[:, :])
            for t in range(4):
                nc.sync.dma_start(out=cws[t], in_=cw_d[t])
            # q10 (scalar): x tiles.
            for t in range(4):
                nc.scalar.dma_start(out=xts[t], in_=xt_d[t])
            nc.vector.memset(ones_row, 1.0)

            # s = style @ (mod_w*lin_scale).T + mod_b   (fp32 matvec)
            for cib in range(4):
                ps_s = preps.tile([128, 1], F32, tag="s")
                for dt_ in range(4):
                    nc.tensor.matmul(ps_s,
                                     modw_t[dt_][:, cib * 128:(cib + 1) * 128],
                                     styl_t[dt_], start=(dt_ == 0), stop=(dt_ == 3))
                nc.vector.scalar_tensor_tensor(
                    out=s_col[:, cib:cib + 1], in0=ps_s, scalar=1.0,
                    in1=modb_t[:, cib:cib + 1], op0=ALU.mult, op1=ALU.add)

            # scale x by s (per-ci partition scalar), in place
            for t in range(4):
                nc.vector.tensor_scalar_mul(xts[t], xts[t], s_col[:, t:t + 1])

            # g[co] = 1/sqrt(c2*A + c2*EPS/wscale^2), A = sum_ci s^2 Q
            nc.scalar.activation(out=s2_t, in_=s_col, func=AF.Square)
            ps_a = preps.tile([1, CO], F32, tag="A")
            for t in range(4):
                nc.tensor.matmul(ps_a, s2_t[:, t:t + 1], qh_t[t],
                                 start=(t == 0), stop=(t == 3))
            c2 = (1.0 + EPS) ** 2
            nc.vector.memset(eps_t, EPS * c2 / (WSCALE * WSCALE))
            nc.scalar.activation(out=sd_row, in_=ps_a, func=AF.Sqrt,
                                 scale=c2, bias=eps_t)
            nc.vector.reciprocal(grow, sd_row)

            # broadcast g and act_b across partitions via rank-1 matmuls
            ps_b = preps.tile([128, CO], F32, tag="bc")
            nc.tensor.matmul(ps_b, ones_row, grow, start=True, stop=True)
            nc.vector.tensor_copy(out=g_b, in_=ps_b)
            ps_b2 = preps.tile([128, CO], F32, tag="bc")
            nc.tensor.matmul(ps_b2, ones_row, actb_t, start=True, stop=True)
            nc.vector.tensor_copy(out=bias_b, in_=ps_b2)

            # ---- phase 1: conv + H-up, u1 streamed to DRAM per w column ----
            # x is [ci, w, h] column-major: lhsT for (j, kh, kw) is one
            # contiguous 128-run covering w-columns {2j+kw, 2j+kw+1}.  Bleed
            # rows land on junk partitions 62/63/126/127, killed by the zero
            # rows of Ulo/Uhi.
            cts = [None] * NB

            def hup(j):
                for dlt in range(2):
                    pu = ps2.tile([128, CO], F32, tag="uh")
                    um = ulo_t if dlt == 0 else uhi_t
                    nc.tensor.matmul(pu, um, cts[j], start=True, stop=True)
                    ut = upool.tile([128, CO], BF16, tag="u1")
                    nc.scalar.copy(out=ut, in_=pu)
                    w = 2 * j + dlt
                    dstw = u1a_d[w] if w < 48 else u1b_d[w - 48]
                    nc.scalar.dma_start(out=dstw, in_=ut)

            def t1_read(gi, b0, b1, w0, w1):
                # ubig[w-part + 64*(hu>=64), hu%64 x co] <- u1[w0:w1, b0:b1, :]
                p0 = (0 if b0 < 64 else 64) + w0
                f0 = b0 % 64
                n = b1 - b0
                dst = ubig[p0:p0 + (w1 - w0), f0 * CO:(f0 + n) * CO]
                dst = dst.rearrange("p (a c) -> p a c", a=n)
                eng = nc.sync if gi % 2 == 0 else nc.scalar
                with nc.allow_non_contiguous_dma(reason="corner turn 1"):
                    src = (u1a_d[w0:w1, b0:b1, :] if w1 <= 48
                           else u1b_d[w0 - 48:w1 - 48, b0:b1, :])
                    eng.dma_start(out=dst, in_=src)

            for j in range(NB):
                pc = ps1.tile([128, CO], F32, tag="conv")
                n_mm = 0
                for t in range(4):
                    for kh in range(KS):
                        for kw in range(KS):
                            rhs = cws[t][:, (kh * 3 + kw) * CO:
                                         (kh * 3 + kw + 1) * CO]
                            base = (2 * j + kw) * 64 + kh
                            lhs = xts[t][:, base:base + 128]
                            nc.tensor.matmul(pc, lhs, rhs,
                                             start=(n_mm == 0),
                                             stop=(n_mm == 35))
                            n_mm += 1
                ct = cpool.tile([128, CO], BF16, tag="c")
                nc.vector.tensor_mul(ct, pc, g_b)
                nc.vector.tensor_add(ct, ct, bias_b)
                cts[j] = ct
                if j >= 1:
                    hup(j - 1)
                if j == 24:
                    # prefetch 77% of corner turn 1 (w < 48 needs only
                    # u1 writes from j <= 23) while conv still runs
                    for gi in range(4):
                        t1_read(gi, 32 * gi, 32 * gi + 32, 0, 48)
            hup(NB - 1)

        # corner turn 1 leftovers (w 48..61), small hu chunks so W-up
        # starts right after the last u1 write
        for gi in range(8):
            t1_read(gi, 16 * gi, 16 * gi + 16, 48, HO)

        # ---- phases 2+3 per co-half m, processed in hu-PAIRS (strided
        # matmul rhs -> N=512 instructions): W-up -> Prelu (Act) -> W-down
        # -> slab8 (DVE) -> d2[m, wd, hu, co] (2KB runs, issues on SP).
        # H-down per 4-wd quad (copy alternates Act/DVE); m=0's H-down is
        # interleaved into m=1's W-stage to hide the turn-2 reads.  Engine
        # budget per pair @2.4GHz: PE 426ns | Act ~prelu 377 | DVE ~slab
        # 320 | SP issues.  Deep buffering (psw 4, a_t 6) keeps the PE fed
        # so it can ramp to max p-state.
        with tc.tile_pool(name="apool", bufs=8) as apool, \
             tc.tile_pool(name="spool", bufs=3) as spool, \
             tc.tile_pool(name="epool", bufs=4) as epool, \
             tc.tile_pool(name="opool", bufs=4) as opool, \
             tc.tile_pool(name="psw", bufs=4, space="PSUM") as psw, \
             tc.tile_pool(name="psd", bufs=2, space="PSUM") as psd, \
             tc.tile_pool(name="psh", bufs=1, space="PSUM") as psh:

            ets = {}

            def t2_reads(m, queues):
                with nc.allow_non_contiguous_dma(reason="corner turn 2"):
                    for c in range(8):
                        et = epool.tile([128, 8 * CQ], BF16, tag="et")
                        src = d2_d[m, 8 * c:8 * c + 8, :, :]
                        queues[c % len(queues)].dma_start(
                            out=et.rearrange("p (a c) -> p a c", a=8),
                            in_=src.transpose([1, 0, 2]))
                        ets[(m, c)] = et

            def hdown(m, wq):
                # one quad = 4 wd columns: 2 matmuls -> 1 copy -> 1 store
                wd = 4 * wq
                ph = psh.tile([64, 4 * CQ], F32, tag="ph")
                et = ets[(m, wd // 8)]
                for i in range(2):
                    rhs = et[:, (wd % 8 + 2 * i) * CQ:(wd % 8 + 2 * i + 2) * CQ]
                    nc.tensor.matmul(ph[:, 2 * i * CQ:(2 * i + 2) * CQ],
                                     dh_t, rhs, start=True, stop=True)
                st = opool.tile([64, 4 * CQ], F32, tag="st")
                if wq % 2 == 0:
                    nc.vector.tensor_copy(out=st, in_=ph)
                else:
                    nc.scalar.copy(out=st, in_=ph)
                with nc.allow_non_contiguous_dma(reason="out store"):
                    nc.sync.dma_start(
                        out=out_d[:, wd:wd + 4, m * CQ:(m + 1) * CQ],
                        in_=st.rearrange("p (a c) -> p a c", a=4))

            pws = [None] * 64
            slabs = [None] * 64

            def wup(m, pr):
                hu = 2 * pr
                k0 = 0 if hu < 64 else 64
                pw = psw.tile([128, 2 * CQ], F32, tag="pw")
                rhs = bass.AP(tensor=ubig.tensor,
                              offset=ubig.offset + k0 * ubig.ap[0][0]
                              + (hu % 64) * CO + m * CQ,
                              ap=[[ubig.ap[0][0], HO], [CO, 2], [1, CQ]])
                nc.tensor.matmul(pw, uw_t[k0:k0 + HO, :], rhs,
                                 start=True, stop=True)
                a_t = apool.tile([128, 2 * CQ], BF16, tag="a")
                nc.scalar.activation(out=a_t, in_=pw, func=AF.Prelu,
                                     alpha=0.2)
                pws[pr] = a_t

            def wdown(m, pr):
                hu = 2 * pr
                pd = psd.tile([64, 2 * CQ], F32, tag="pd")
                nc.tensor.matmul(pd, dw_t, pws[pr], start=True, stop=True)
                if pr % 4 == 0:
                    slab = spool.tile([64, 8 * CQ], BF16, tag="sl")
                    slabs[pr] = slab
                else:
                    slab = slabs[pr - pr % 4]
                nc.vector.tensor_copy(
                    out=slab[:, (pr % 4) * 2 * CQ:(pr % 4 + 1) * 2 * CQ],
                    in_=pd)
                if pr % 4 == 3:
                    with nc.allow_non_contiguous_dma(reason="d2 store"):
                        nc.sync.dma_start(
                            out=d2_d[m, :, hu - 6:hu + 2, :],
                            in_=slab.rearrange("p (a c) -> p a c", a=8))

            # W-stage m=0 (software-pipelined by 3 pairs)
            for pr in range(64):
                wup(0, pr)
                if pr >= 3:
                    wdown(0, pr - 3)
            for pr in range(61, 64):
                wdown(0, pr)

            # T2 reads for m=0 queue up behind the last d2 write
            t2_reads(0, [nc.sync])

            # W-stage m=1 with m=0's H-down interleaved
            hd_done = 0
            for pr in range(64):
                wup(1, pr)
                if pr >= 3:
                    wdown(1, pr - 3)
                want = 0 if pr < 8 else min(16, 1 + (pr - 8) * 15 // 55)
                while hd_done < want:
                    hdown(0, hd_done)
                    hd_done += 1
            for pr in range(61, 64):
                wdown(1, pr)
            while hd_done < 16:
                hdown(0, hd_done)
                hd_done += 1

            t2_reads(1, [nc.sync, nc.scalar])
            for wq in range(16):
                hdown(1, wq)

    nc.compile()
    return nc


def _host_prep(x, style, mod_w, mod_b, conv_w, act_b, up_filter, down_filter):
    import ml_dtypes
    bf16 = ml_dtypes.bfloat16
    x = np.asarray(x, np.float32)
    style = np.asarray(style, np.float32)
    mod_w = np.asarray(mod_w, np.float32)
    mod_b = np.asarray(mod_b, np.float32)
    conv_w = np.asarray(conv_w, np.float32)
    act_b = np.asarray(act_b, np.float32)
    up_filter = np.asarray(up_filter, np.float64)
    down_filter = np.asarray(down_filter, np.float64)

    # FIR matrices (see upfirdn2d in the reference):
    #   up:   y[o] = sum_i fu[o + 3 - 2i] x[i],   fu = up_filter * 2
    #   down: y[o] = sum_u df[2o + 6 - u] x[u]
    fu = up_filter * UP
    U = np.zeros((HO, 2 * H), np.float32)
    for i in range(HO):
        for o in range(2 * H):
            t = o + 3 - 2 * i
            if 0 <= t < TAPS:
                U[i, o] = fu[t]
    D = np.zeros((2 * H, H), np.float32)
    for u in range(2 * H):
        for o in range(H):
            t = 2 * o + 6 - u
            if 0 <= t < TAPS:
                D[u, o] = down_filter[t]
    ulo = np.zeros((128, 128), np.float32)
    uhi = np.zeros((128, 128), np.float32)
    uw = np.zeros((128, 128), np.float32)
    ulo[0:HO, :] = U
    uhi[64:64 + HO, :] = U
    uw[0:HO, :] = U
    uw[64:64 + HO, :] = U
    dw = SQRT2 * D          # sqrt2 lrelu gain folded into W-down
    dh = D

    modw_host = np.ascontiguousarray(
        (mod_w * LIN_SCALE).T.reshape(4, 128, 512), np.float32)
    modb_host = np.ascontiguousarray(mod_b.reshape(4, 128).T, np.float32)
    qh_host = np.ascontiguousarray(
        (conv_w.astype(np.float64) ** 2).sum(axis=(2, 3)).T
        .reshape(4, 128, CO), np.float32)
    cw_host = np.ascontiguousarray(
        conv_w.transpose(1, 2, 3, 0).reshape(4, 128, K9 * CO)).astype(bf16)
    actb_host = np.ascontiguousarray(act_b.reshape(1, CO), np.float32)

    # column-major x: [ci, w, h], flat + 128 zero pad at the end
    xp = np.zeros((B, 4, 128, XPAD), bf16)
    xcm = x.reshape(B, 4, 128, 64, 64).transpose(0, 1, 2, 4, 3)  # [.., w, h]
    xp[:, :, :, 0:64 * 64] = xcm.reshape(B, 4, 128, 64 * 64).astype(bf16)
    stylec = np.ascontiguousarray(style.reshape(B, 4, 128, 1), np.float32)

    shared = {
        "modw": modw_host, "modb": modb_host, "qh": qh_host, "cw": cw_host,
        "actb": actb_host, "ulo": ulo.astype(bf16), "uhi": uhi.astype(bf16),
        "uw": uw.astype(bf16), "dw": dw.astype(bf16), "dh": dh.astype(bf16),
    }
    in_maps = []
    for b in range(B):
        im = dict(shared)
        im["xt"] = np.ascontiguousarray(xp[b])
        im["stylec"] = stylec[b]
        in_maps.append(im)
    return in_maps


def kernel(**inputs):
    _install_neff_cache()
    if "nc" not in _CACHE:
        _CACHE["nc"] = _build_nc()
    nc = _CACHE["nc"]
    in_maps = _host_prep(**inputs)
    trace = os.environ.get("AFC_TRACE", "0") == "1"
    res = run_bass_kernel_spmd(nc, in_maps, list(range(B)), trace=trace)
    _CACHE["last_result"] = res
    out = np.stack([r["o"].transpose(2, 0, 1) for r in res.results])
    return np.ascontiguousarray(out, np.float32)


def _install_neff_cache():
    """Disk-cache walrus compiles by BIR hash (compile is ~10 min)."""
    import hashlib
    import shutil as _sh
    from concourse import bass_utils as _bu
    from concourse import bass2jax as _bj
    if getattr(_bu, "_afc_cache_installed", False):
        return
    orig = _bu.compile_bir_kernel
    cache_dir = "/tmp/afc_neff_cache"
    os.makedirs(cache_dir, exist_ok=True)

    def cached(bir_json, tmpdir, neff_name="file.neff"):
        data = bir_json if isinstance(bir_json, bytes) else bir_json.encode()
        h = hashlib.sha256(data).hexdigest()[:24]
        cpath = os.path.join(cache_dir, h + ".neff")
        dst = os.path.join(tmpdir, neff_name)
        if os.path.exists(cpath):
            _sh.copy(cpath, dst)
            return dst
        p = orig(bir_json, tmpdir, neff_name)
        try:
            _sh.copy(p, cpath)
        except OSError:
            pass
        return p

    _bu.compile_bir_kernel = cached
    _bj.compile_bir_kernel = cached
    _bu._afc_cache_installed = True


def _make_runner(nc, in_maps, k=1):
    """Build a reusable jitted shard_map callable over 8 cores with
    device-resident inputs (mirrors bass2jax.run_bass_via_pjrt). With k>1
    the NEFF executes k times per dispatch, chained through the donated
    output operands so XLA cannot CSE or parallelize the calls."""
    import jax
    from jax.experimental.shard_map import shard_map
    from jax.sharding import Mesh, NamedSharding, PartitionSpec
    from concourse import bass2jax

    bass2jax.install_neuronx_cc_hook()
    partition_name = nc.partition_id_tensor.name if nc.partition_id_tensor else None
    in_names, out_names, out_avals, zero_outs = [], [], [], []
    for alloc in nc.m.functions[0].allocations:
        if not isinstance(alloc, mybir.MemoryLocationSet):
            continue
        name = alloc.memorylocations[0].name
        if alloc.kind == "ExternalInput":
            if name != partition_name:
                in_names.append(name)
        elif alloc.kind == "ExternalOutput":
            out_names.append(name)
            shape = tuple(alloc.tensor_shape)
            dtype = mybir.dt.np(alloc.dtype)
            out_avals.append(jax.core.ShapedArray(shape, dtype))
            zero_outs.append(np.zeros(shape, dtype))
    n_params = len(in_names)
    all_names = list(in_names) + out_names
    if partition_name is not None:
        all_names.append(partition_name)

    def _body(*args):
        ins = list(args[:n_params])
        outs = list(args[n_params:])
        for _ in range(k):
            operands = ins + outs
            if partition_name is not None:
                operands.append(bass2jax.partition_id_tensor())
            outs = list(bass2jax._bass_exec_p.bind(
                *operands, out_avals=tuple(out_avals),
                in_names=tuple(all_names), out_names=tuple(out_names),
                lowering_input_output_aliases=(), sim_require_finite=True,
                sim_require_nnan=True, nc=nc))
        return tuple(outs)

    n = len(in_maps)
    devices = jax.devices()[:n]
    mesh = Mesh(np.asarray(devices), ("core",))
    nin = n_params + len(out_names)
    f = jax.jit(shard_map(_body, mesh=mesh,
                          in_specs=(PartitionSpec("core"),) * nin,
                          out_specs=(PartitionSpec("core"),) * len(out_names),
                          check_rep=False), keep_unused=True)
    sh = NamedSharding(mesh, PartitionSpec("core"))
    args = [jax.device_put(
        np.concatenate([np.asarray(m[nm]) for m in in_maps], axis=0), sh)
        for nm in in_names]
    args += [jax.device_put(
        np.zeros((n * z.shape[0], *z.shape[1:]), z.dtype), sh)
        for z in zero_outs]
    return f, args


def _time_runner(f, args, iters):
    import time as _time
    for _ in range(2):
        jax.block_until_ready(f(*args))
    best = float("inf")
    for _ in range(iters):
        t0 = _time.perf_counter()
        jax.block_until_ready(f(*args))
        best = min(best, _time.perf_counter() - t0)
    return best


def time_kernel(iters=6, k=24, **inputs):
    """Per-execution time via an in-kernel For_i repeat loop: the whole
    pipeline runs 1x and (1+k)x per dispatch; the difference isolates
    device time from the ~80 ms axon dispatch overhead. Returns ns."""
    global jax
    import jax
    _install_neff_cache()
    if "nc" not in _CACHE:
        _CACHE["nc"] = _build_nc()
    in_maps = _host_prep(**inputs)
    nck = _build_nc(reps=1 + k)
    f1, args = _make_runner(_CACHE["nc"], in_maps)
    fk, _ = _make_runner(nck, in_maps)
    t1 = _time_runner(f1, args, iters)
    tk = _time_runner(fk, args, iters)
    print(f"wall 1x: {t1*1e3:.2f} ms, {1+k}x: {tk*1e3:.2f} ms")
    return (tk - t1) / k * 1e9


# revision 13
# speedup vs baseline: 1.0323x; 1.0323x over previous
"""AliasFreeConv Trainium2 kernel (bf16 pipeline, DRAM corner turns).

Data-parallel over batch: 8 samples -> 8 NeuronCores, no collectives.
Per core, all heavy math in bf16 (PE 1 cycle/row, rel-err ~3e-3 vs the
2e-2 gate):

  phase 0: style matvec (PE, fp32) -> s[ci]; x scaled by s in-place;
           demod g[co] = wscale*rsqrt(wscale^2 * s^2.Q + eps)/(1+eps)
           via host-precomputed Q[ci,co] = sum_k conv_w^2.
  phase 1: per w-column-pair j: 3x3 VALID conv as 36 shifted bf16
           matmuls -> psum; ct = psum*g + bias (DVE); H-up via Ulo/Uhi
           [128x128] matmuls (junk rows killed by zero filter rows);
           u1[w] = [128 hu, 512 co] bf16 -> DRAM (contiguous).
  phase 2: corner turn #1: chunked strided reads u1 -> ubig[w-part,
           hu x co] (hu<64 at partitions 0.., hu>=64 at 64..);
           per (m=co-half, hu): W-up matmul (K=62) -> Prelu (alternating
           ACT/DVE, sqrt2 folded into W-down matrix) -> W-down -> slab
           -> d2[hu] DRAM (corner turn #2).
  phase 3: chunked strided reads d2 -> et[hu-part, wd x co]; H-down
           matmuls -> out stores. m=0 H-down interleaved into m=1's
           W-stage to hide the turn-2 read latency.

Corner turns bounce through DRAM because SBUF->SBUF scatters into few
partitions run at ~20 GB/s (measured), while DRAM strided reads with
>=512B runs sustain ~190 GB/s and contiguous writes ~390 GB/s.
"""
import math
import os
import numpy as np
from contextlib import ExitStack

import concourse.bass as bass
import concourse.bacc as bacc
import concourse.tile as tile
from concourse import mybir
from concourse.bass_utils import run_bass_kernel_spmd

F32 = mybir.dt.float32
BF16 = mybir.dt.bfloat16
AF = mybir.ActivationFunctionType
ALU = mybir.AluOpType

B, CI, CO, H, W = 8, 512, 512, 64, 64
KS, TAPS, UP = 3, 12, 2
HO = H - 2                      # 62 valid conv outputs per axis
STYLE = 512
XPAD = 66 * 64                  # x tile free size: 64 cols x 66 rows
NB = HO // 2                    # 31 w-column-pair blocks
K9 = KS * KS
LIN_SCALE = 1.0 / math.sqrt(STYLE)
WSCALE = 1.0 / math.sqrt(CI * KS * KS)
EPS = 1e-8
SQRT2 = math.sqrt(2.0)
CQ = 256                        # co half for phases 2/3

_CACHE: dict = {}


def _build_nc(reps=1):
    nc = bacc.Bacc()

    xt_d = nc.declare_dram_parameter("xt", [4, 128, XPAD], BF16, isOutput=False)
    stylec_d = nc.declare_dram_parameter("stylec", [4, 128, 1], F32, isOutput=False)
    modw_d = nc.declare_dram_parameter("modw", [4, 128, 512], F32, isOutput=False)
    modb_d = nc.declare_dram_parameter("modb", [128, 4], F32, isOutput=False)
    qh_d = nc.declare_dram_parameter("qh", [4, 128, CO], F32, isOutput=False)
    cw_d = nc.declare_dram_parameter("cw", [4, 128, K9 * CO], BF16, isOutput=False)
    actb_d = nc.declare_dram_parameter("actb", [1, CO], F32, isOutput=False)
    ulo_d = nc.declare_dram_parameter("ulo", [128, 128], BF16, isOutput=False)
    uhi_d = nc.declare_dram_parameter("uhi", [128, 128], BF16, isOutput=False)
    uw_d = nc.declare_dram_parameter("uw", [128, 128], BF16, isOutput=False)
    dw_d = nc.declare_dram_parameter("dw", [128, 64], BF16, isOutput=False)
    dh_d = nc.declare_dram_parameter("dh", [128, 64], BF16, isOutput=False)
    out_d = nc.declare_dram_parameter("o", [64, 64, CO], F32, isOutput=True)

    u1_d = nc.dram_tensor("u1scratch", [HO, 128, CO], BF16)   # [w, hu, co]
    d2_d = nc.dram_tensor("d2scratch", [2, 64, 128, CQ], BF16)  # [m, wd, hu, co]

    with ExitStack() as ctx:
        tc = ctx.enter_context(tile.TileContext(nc))
        if reps > 1:
            ctx.enter_context(tc.For_i(0, reps, 1))
        pp = ctx.enter_context(tc.tile_pool(name="persist", bufs=1))

        ulo_t = pp.tile([128, 128], BF16)
        uhi_t = pp.tile([128, 128], BF16)
        uw_t = pp.tile([128, 128], BF16)
        dw_t = pp.tile([128, 64], BF16)
        dh_t = pp.tile([128, 64], BF16)
        g_b = pp.tile([128, CO], F32)
        bias_b = pp.tile([128, CO], F32)
        s_col = pp.tile([128, 4], F32)
        ubig = pp.tile([128, 64 * CO], BF16)     # [w(+64 for hu>=64), hu%64 x co]

        with tc.tile_pool(name="xw", bufs=1) as xw, \
             tc.tile_pool(name="pre", bufs=1) as pre, \
             tc.tile_pool(name="preps", bufs=1, space="PSUM") as preps, \
             tc.tile_pool(name="cpool", bufs=3) as cpool, \
             tc.tile_pool(name="upool", bufs=3) as upool, \
             tc.tile_pool(name="ps1", bufs=2, space="PSUM") as ps1, \
             tc.tile_pool(name="ps2", bufs=2, space="PSUM") as ps2:

            xts = [xw.tile([128, XPAD], BF16, tag=f"x{t}", name=f"x{t}") for t in range(4)]
            cws = [xw.tile([128, K9 * CO], BF16, tag=f"w{t}", name=f"w{t}") for t in range(4)]
            modw_t = [pre.tile([128, 512], F32, tag=f"mw{t}", name=f"mw{t}") for t in range(4)]
            styl_t = [pre.tile([128, 1], F32, tag=f"st{t}", name=f"st{t}") for t in range(4)]
            qh_t = [pre.tile([128, CO], F32, tag=f"qh{t}", name=f"qh{t}") for t in range(4)]
            modb_t = pre.tile([128, 4], F32)
            actb_t = pre.tile([1, CO], F32)
            ones_row = pre.tile([1, 128], F32)
            s2_t = pre.tile([128, 4], F32)
            sd_row = pre.tile([1, CO], F32)
            grow = pre.tile([1, CO], F32)
            eps_t = pre.tile([1, 1], F32)

            # q1 (sync): small prologue tensors first, then conv weights.
            for t in range(4):
                nc.sync.dma_start(out=modw_t[t], in_=modw_d[t])
                nc.sync.dma_start(out=styl_t[t], in_=stylec_d[t])
            nc.sync.dma_start(out=modb_t, in_=modb_d[:, :])
            nc.sync.dma_start(out=ulo_t, in_=ulo_d[:, :])
            nc.sync.dma_start(out=uhi_t, in_=uhi_d[:, :])
            nc.sync.dma_start(out=uw_t, in_=uw_d[:, :])
            nc.sync.dma_start(out=dw_t, in_=dw_d[:, :])
            nc.sync.dma_start(out=dh_t, in_=dh_d[:, :])
            for t in range(4):
                nc.sync.dma_start(out=qh_t[t], in_=qh_d[t])
            nc.sync.dma_start(out=actb_t, in_=actb_d[:, :])
            for t in range(4):
                nc.sync.dma_start(out=cws[t], in_=cw_d[t])
            # q10 (scalar): x tiles.
            for t in range(4):
                nc.scalar.dma_start(out=xts[t], in_=xt_d[t])
            nc.vector.memset(ones_row, 1.0)

            # s = style @ (mod_w*lin_scale).T + mod_b   (fp32 matvec)
            for cib in range(4):
                ps_s = preps.tile([128, 1], F32, tag="s")
                for dt_ in range(4):
                    nc.tensor.matmul(ps_s,
                                     modw_t[dt_][:, cib * 128:(cib + 1) * 128],
                                     styl_t[dt_], start=(dt_ == 0), stop=(dt_ == 3))
                nc.vector.scalar_tensor_tensor(
                    out=s_col[:, cib:cib + 1], in0=ps_s, scalar=1.0,
                    in1=modb_t[:, cib:cib + 1], op0=ALU.mult, op1=ALU.add)

            # scale x by s (per-ci partition scalar), in place
            for t in range(4):
                nc.vector.tensor_scalar_mul(xts[t], xts[t], s_col[:, t:t + 1])

            # g[co] = 1/sqrt(c2*A + c2*EPS/wscale^2), A = sum_ci s^2 Q
            nc.scalar.activation(out=s2_t, in_=s_col, func=AF.Square)
            ps_a = preps.tile([1, CO], F32, tag="A")
            for t in range(4):
                nc.tensor.matmul(ps_a, s2_t[:, t:t + 1], qh_t[t],
                                 start=(t == 0), stop=(t == 3))
            c2 = (1.0 + EPS) ** 2
            nc.vector.memset(eps_t, EPS * c2 / (WSCALE * WSCALE))
            nc.scalar.activation(out=sd_row, in_=ps_a, func=AF.Sqrt,
                                 scale=c2, bias=eps_t)
            nc.vector.reciprocal(grow, sd_row)

            # broadcast g and act_b across partitions via rank-1 matmuls
            ps_b = preps.tile([128, CO], F32, tag="bc")
            nc.tensor.matmul(ps_b, ones_row, grow, start=True, stop=True)
            nc.vector.tensor_copy(out=g_b, in_=ps_b)
            ps_b2 = preps.tile([128, CO], F32, tag="bc")
            nc.tensor.matmul(ps_b2, ones_row, actb_t, start=True, stop=True)
            nc.vector.tensor_copy(out=bias_b, in_=ps_b2)

            # ---- phase 1: conv + H-up, u1 streamed to DRAM per w column ----
            # x is [ci, w, h] column-major: lhsT for (j, kh, kw) is one
            # contiguous 128-run covering w-columns {2j+kw, 2j+kw+1}.  Bleed
            # rows land on junk partitions 62/63/126/127, killed by the zero
            # rows of Ulo/Uhi.
            cts = [None] * NB

            def hup(j):
                for dlt in range(2):
                    pu = ps2.tile([128, CO], F32, tag="uh")
                    um = ulo_t if dlt == 0 else uhi_t
                    nc.tensor.matmul(pu, um, cts[j], start=True, stop=True)
                    ut = upool.tile([128, CO], BF16, tag="u1")
                    nc.scalar.copy(out=ut, in_=pu)
                    nc.scalar.dma_start(out=u1_d[2 * j + dlt], in_=ut)

            def t1_read(gi, b0, b1, w0, w1):
                # ubig[w-part + 64*(hu>=64), hu%64 x co] <- u1[w0:w1, b0:b1, :]
                p0 = (0 if b0 < 64 else 64) + w0
                f0 = b0 % 64
                n = b1 - b0
                dst = ubig[p0:p0 + (w1 - w0), f0 * CO:(f0 + n) * CO]
                dst = dst.rearrange("p (a c) -> p a c", a=n)
                eng = nc.sync if gi % 2 == 0 else nc.scalar
                with nc.allow_non_contiguous_dma(reason="corner turn 1"):
                    eng.dma_start(out=dst, in_=u1_d[w0:w1, b0:b1, :])

            for j in range(NB):
                pc = ps1.tile([128, CO], F32, tag="conv")
                n_mm = 0
                for t in range(4):
                    for kh in range(KS):
                        for kw in range(KS):
                            rhs = cws[t][:, (kh * 3 + kw) * CO:
                                         (kh * 3 + kw + 1) * CO]
                            base = (2 * j + kw) * 64 + kh
                            lhs = xts[t][:, base:base + 128]
                            nc.tensor.matmul(pc, lhs, rhs,
                                             start=(n_mm == 0),
                                             stop=(n_mm == 35))
                            n_mm += 1
                ct = cpool.tile([128, CO], BF16, tag="c")
                nc.vector.tensor_mul(ct, pc, g_b)
                nc.vector.tensor_add(ct, ct, bias_b)
                cts[j] = ct
                if j >= 1:
                    hup(j - 1)
                if j == 24:
                    # prefetch 77% of corner turn 1 (w < 48 needs only
                    # u1 writes from j <= 23) while conv still runs
                    for gi in range(4):
                        t1_read(gi, 32 * gi, 32 * gi + 32, 0, 48)
            hup(NB - 1)

        # corner turn 1 leftovers (w 48..61), small hu chunks so W-up
        # starts right after the last u1 write
        for gi in range(8):
            t1_read(gi, 16 * gi, 16 * gi + 16, 48, HO)

        # ---- phases 2+3 per co-half m, processed in hu-PAIRS (strided
        # matmul rhs -> N=512 instructions): W-up -> Prelu (Act) -> W-down
        # -> slab8 (DVE) -> d2[m, wd, hu, co] (2KB runs, issues on SP).
        # H-down per 4-wd quad (copy alternates Act/DVE); m=0's H-down is
        # interleaved into m=1's W-stage to hide the turn-2 reads.  Engine
        # budget per pair @2.4GHz: PE 426ns | Act ~prelu 377 | DVE ~slab
        # 320 | SP issues.  Deep buffering (psw 4, a_t 6) keeps the PE fed
        # so it can ramp to max p-state.
        with tc.tile_pool(name="apool", bufs=6) as apool, \
             tc.tile_pool(name="spool", bufs=3) as spool, \
             tc.tile_pool(name="epool", bufs=4) as epool, \
             tc.tile_pool(name="opool", bufs=4) as opool, \
             tc.tile_pool(name="psw", bufs=4, space="PSUM") as psw, \
             tc.tile_pool(name="psd", bufs=2, space="PSUM") as psd, \
             tc.tile_pool(name="psh", bufs=1, space="PSUM") as psh:

            ets = {}

            def t2_reads(m, queues):
                with nc.allow_non_contiguous_dma(reason="corner turn 2"):
                    for c in range(8):
                        et = epool.tile([128, 8 * CQ], BF16, tag="et")
                        src = d2_d[m, 8 * c:8 * c + 8, :, :]
                        queues[c % len(queues)].dma_start(
                            out=et.rearrange("p (a c) -> p a c", a=8),
                            in_=src.transpose([1, 0, 2]))
                        ets[(m, c)] = et

            def hdown(m, wq):
                # one quad = 4 wd columns: 2 matmuls -> 1 copy -> 1 store
                wd = 4 * wq
                ph = psh.tile([64, 4 * CQ], F32, tag="ph")
                et = ets[(m, wd // 8)]
                for i in range(2):
                    rhs = et[:, (wd % 8 + 2 * i) * CQ:(wd % 8 + 2 * i + 2) * CQ]
                    nc.tensor.matmul(ph[:, 2 * i * CQ:(2 * i + 2) * CQ],
                                     dh_t, rhs, start=True, stop=True)
                st = opool.tile([64, 4 * CQ], F32, tag="st")
                if wq % 2 == 0:
                    nc.vector.tensor_copy(out=st, in_=ph)
                else:
                    nc.scalar.copy(out=st, in_=ph)
                with nc.allow_non_contiguous_dma(reason="out store"):
                    nc.sync.dma_start(
                        out=out_d[:, wd:wd + 4, m * CQ:(m + 1) * CQ],
                        in_=st.rearrange("p (a c) -> p a c", a=4))

            pws = [None] * 64
            slabs = [None] * 64

            def wup(m, pr):
                hu = 2 * pr
                k0 = 0 if hu < 64 else 64
                pw = psw.tile([128, 2 * CQ], F32, tag="pw")
                rhs = bass.AP(tensor=ubig.tensor,
                              offset=ubig.offset + k0 * ubig.ap[0][0]
                              + (hu % 64) * CO + m * CQ,
                              ap=[[ubig.ap[0][0], HO], [CO, 2], [1, CQ]])
                nc.tensor.matmul(pw, uw_t[k0:k0 + HO, :], rhs,
                                 start=True, stop=True)
                a_t = apool.tile([128, 2 * CQ], BF16, tag="a")
                nc.scalar.activation(out=a_t, in_=pw, func=AF.Prelu,
                                     alpha=0.2)
                pws[pr] = a_t

            def wdown(m, pr):
                hu = 2 * pr
                pd = psd.tile([64, 2 * CQ], F32, tag="pd")
                nc.tensor.matmul(pd, dw_t, pws[pr], start=True, stop=True)
                if pr % 4 == 0:
                    slab = spool.tile([64, 8 * CQ], BF16, tag="sl")
                    slabs[pr] = slab
                else:
                    slab = slabs[pr - pr % 4]
                nc.vector.tensor_copy(
                    out=slab[:, (pr % 4) * 2 * CQ:(pr % 4 + 1) * 2 * CQ],
                    in_=pd)
                if pr % 4 == 3:
                    with nc.allow_non_contiguous_dma(reason="d2 store"):
                        nc.sync.dma_start(
                            out=d2_d[m, :, hu - 6:hu + 2, :],
                            in_=slab.rearrange("p (a c) -> p a c", a=8))

            # W-stage m=0 (software-pipelined by 3 pairs)
            for pr in range(64):
                wup(0, pr)
                if pr >= 3:
                    wdown(0, pr - 3)
            for pr in range(61, 64):
                wdown(0, pr)

            # T2 reads for m=0 queue up behind the last d2 write
            t2_reads(0, [nc.sync])

            # W-stage m=1 with m=0's H-down interleaved
            hd_done = 0
            for pr in range(64):
                wup(1, pr)
                if pr >= 3:
                    wdown(1, pr - 3)
                want = 0 if pr < 8 else min(16, 1 + (pr - 8) * 15 // 55)
                while hd_done < want:
                    hdown(0, hd_done)
                    hd_done += 1
            for pr in range(61, 64):
                wdown(1, pr)
            while hd_done < 16:
                hdown(0, hd_done)
                hd_done += 1

            t2_reads(1, [nc.sync, nc.scalar])
            for wq in range(16):
                hdown(1, wq)

    nc.compile()
    return nc


def _host_prep(x, style, mod_w, mod_b, conv_w, act_b, up_filter, down_filter):
    import ml_dtypes
    bf16 = ml_dtypes.bfloat16
    x = np.asarray(x, np.float32)
    style = np.asarray(style, np.float32)
    mod_w = np.asarray(mod_w, np.float32)
    mod_b = np.asarray(mod_b, np.float32)
    conv_w = np.asarray(conv_w, np.float32)
    act_b = np.asarray(act_b, np.float32)
    up_filter = np.asarray(up_filter, np.float64)
    down_filter = np.asarray(down_filter, np.float64)

    # FIR matrices (see upfirdn2d in the reference):
    #   up:   y[o] = sum_i fu[o + 3 - 2i] x[i],   fu = up_filter * 2
    #   down: y[o] = sum_u df[2o + 6 - u] x[u]
    fu = up_filter * UP
    U = np.zeros((HO, 2 * H), np.float32)
    for i in range(HO):
        for o in range(2 * H):
            t = o + 3 - 2 * i
            if 0 <= t < TAPS:
                U[i, o] = fu[t]
    D = np.zeros((2 * H, H), np.float32)
    for u in range(2 * H):
        for o in range(H):
            t = 2 * o + 6 - u
            if 0 <= t < TAPS:
                D[u, o] = down_filter[t]
    ulo = np.zeros((128, 128), np.float32)
    uhi = np.zeros((128, 128), np.float32)
    uw = np.zeros((128, 128), np.float32)
    ulo[0:HO, :] = U
    uhi[64:64 + HO, :] = U
    uw[0:HO, :] = U
    uw[64:64 + HO, :] = U
    dw = SQRT2 * D          # sqrt2 lrelu gain folded into W-down
    dh = D

    modw_host = np.ascontiguousarray(
        (mod_w * LIN_SCALE).T.reshape(4, 128, 512), np.float32)
    modb_host = np.ascontiguousarray(mod_b.reshape(4, 128).T, np.float32)
    qh_host = np.ascontiguousarray(
        (conv_w.astype(np.float64) ** 2).sum(axis=(2, 3)).T
        .reshape(4, 128, CO), np.float32)
    cw_host = np.ascontiguousarray(
        conv_w.transpose(1, 2, 3, 0).reshape(4, 128, K9 * CO)).astype(bf16)
    actb_host = np.ascontiguousarray(act_b.reshape(1, CO), np.float32)

    # column-major x: [ci, w, h], flat + 128 zero pad at the end
    xp = np.zeros((B, 4, 128, XPAD), bf16)
    xcm = x.reshape(B, 4, 128, 64, 64).transpose(0, 1, 2, 4, 3)  # [.., w, h]
    xp[:, :, :, 0:64 * 64] = xcm.reshape(B, 4, 128, 64 * 64).astype(bf16)
    stylec = np.ascontiguousarray(style.reshape(B, 4, 128, 1), np.float32)

    shared = {
        "modw": modw_host, "modb": modb_host, "qh": qh_host, "cw": cw_host,
        "actb": actb_host, "ulo": ulo.astype(bf16), "uhi": uhi.astype(bf16),
        "uw": uw.astype(bf16), "dw": dw.astype(bf16), "dh": dh.astype(bf16),
    }
    in_maps = []
    for b in range(B):
        im = dict(shared)
        im["xt"] = np.ascontiguousarray(xp[b])
        im["stylec"] = stylec[b]
        in_maps.append(im)
    return in_maps


def kernel(**inputs):
    _install_neff_cache()
    if "nc" not in _CACHE:
        _CACHE["nc"] = _build_nc()
    nc = _CACHE["nc"]
    in_maps = _host_prep(**inputs)
    trace = os.environ.get("AFC_TRACE", "0") == "1"
    res = run_bass_kernel_spmd(nc, in_maps, list(range(B)), trace=trace)
    _CACHE["last_result"] = res
    out = np.stack([r["o"].transpose(2, 0, 1) for r in res.results])
    return np.ascontiguousarray(out, np.float32)


def _install_neff_cache():
    """Disk-cache walrus compiles by BIR hash (compile is ~10 min)."""
    import hashlib
    import shutil as _sh
    from concourse import bass_utils as _bu
    from concourse import bass2jax as _bj
    if getattr(_bu, "_afc_cache_installed", False):
        return
    orig = _bu.compile_bir_kernel
    cache_dir = "/tmp/afc_neff_cache"
    os.makedirs(cache_dir, exist_ok=True)

    def cached(bir_json, tmpdir, neff_name="file.neff"):
        data = bir_json if isinstance(bir_json, bytes) else bir_json.encode()
        h = hashlib.sha256(data).hexdigest()[:24]
        cpath = os.path.join(cache_dir, h + ".neff")
        dst = os.path.join(tmpdir, neff_name)
        if os.path.exists(cpath):
            _sh.copy(cpath, dst)
            return dst
        p = orig(bir_json, tmpdir, neff_name)
        try:
            _sh.copy(p, cpath)
        except OSError:
            pass
        return p

    _bu.compile_bir_kernel = cached
    _bj.compile_bir_kernel = cached
    _bu._afc_cache_installed = True


def _make_runner(nc, in_maps, k=1):
    """Build a reusable jitted shard_map callable over 8 cores with
    device-resident inputs (mirrors bass2jax.run_bass_via_pjrt). With k>1
    the NEFF executes k times per dispatch, chained through the donated
    output operands so XLA cannot CSE or parallelize the calls."""
    import jax
    from jax.experimental.shard_map import shard_map
    from jax.sharding import Mesh, NamedSharding, PartitionSpec
    from concourse import bass2jax

    bass2jax.install_neuronx_cc_hook()
    partition_name = nc.partition_id_tensor.name if nc.partition_id_tensor else None
    in_names, out_names, out_avals, zero_outs = [], [], [], []
    for alloc in nc.m.functions[0].allocations:
        if not isinstance(alloc, mybir.MemoryLocationSet):
            continue
        name = alloc.memorylocations[0].name
        if alloc.kind == "ExternalInput":
            if name != partition_name:
                in_names.append(name)
        elif alloc.kind == "ExternalOutput":
            out_names.append(name)
            shape = tuple(alloc.tensor_shape)
            dtype = mybir.dt.np(alloc.dtype)
            out_avals.append(jax.core.ShapedArray(shape, dtype))
            zero_outs.append(np.zeros(shape, dtype))
    n_params = len(in_names)
    all_names = list(in_names) + out_names
    if partition_name is not None:
        all_names.append(partition_name)

    def _body(*args):
        ins = list(args[:n_params])
        outs = list(args[n_params:])
        for _ in range(k):
            operands = ins + outs
            if partition_name is not None:
                operands.append(bass2jax.partition_id_tensor())
            outs = list(bass2jax._bass_exec_p.bind(
                *operands, out_avals=tuple(out_avals),
                in_names=tuple(all_names), out_names=tuple(out_names),
                lowering_input_output_aliases=(), sim_require_finite=True,
                sim_require_nnan=True, nc=nc))
        return tuple(outs)

    n = len(in_maps)
    devices = jax.devices()[:n]
    mesh = Mesh(np.asarray(devices), ("core",))
    nin = n_params + len(out_names)
    f = jax.jit(shard_map(_body, mesh=mesh,
                          in_specs=(PartitionSpec("core"),) * nin,
                          out_specs=(PartitionSpec("core"),) * len(out_names),
                          check_rep=False), keep_unused=True)
    sh = NamedSharding(mesh, PartitionSpec("core"))
    args = [jax.device_put(
        np.concatenate([np.asarray(m[nm]) for m in in_maps], axis=0), sh)
        for nm in in_names]
    args += [jax.device_put(
        np.zeros((n * z.shape[0], *z.shape[1:]), z.dtype), sh)
        for z in zero_outs]
    return f, args


def _time_runner(f, args, iters):
    import time as _time
    for _ in range(2):
        jax.block_until_ready(f(*args))
    best = float("inf")
    for _ in range(iters):
        t0 = _time.perf_counter()
        jax.block_until_ready(f(*args))
        best = min(best, _time.perf_counter() - t0)
    return best


def time_kernel(iters=6, k=24, **inputs):
    """Per-execution time via an in-kernel For_i repeat loop: the whole
    pipeline runs 1x and (1+k)x per dispatch; the difference isolates
    device time from the ~80 ms axon dispatch overhead. Returns ns."""
    global jax
    import jax
    _install_neff_cache()
    if "nc" not in _CACHE:
        _CACHE["nc"] = _build_nc()
    in_maps = _host_prep(**inputs)
    nck = _build_nc(reps=1 + k)
    f1, args = _make_runner(_CACHE["nc"], in_maps)
    fk, _ = _make_runner(nck, in_maps)
    t1 = _time_runner(f1, args, iters)
    tk = _time_runner(fk, args, iters)
    print(f"wall 1x: {t1*1e3:.2f} ms, {1+k}x: {tk*1e3:.2f} ms")
    return (tk - t1) / k * 1e9


# revision 14
# speedup vs baseline: 1.1157x; 1.0807x over previous
"""AliasFreeConv Trainium2 kernel (bf16 pipeline, DRAM corner turns).

Data-parallel over batch: 8 samples -> 8 NeuronCores, no collectives.
Per core, all heavy math in bf16 (PE 1 cycle/row, rel-err ~3e-3 vs the
2e-2 gate):

  phase 0: style matvec (PE, fp32) -> s[ci]; x scaled by s in-place;
           demod g[co] = wscale*rsqrt(wscale^2 * s^2.Q + eps)/(1+eps)
           via host-precomputed Q[ci,co] = sum_k conv_w^2.
  phase 1: per w-column-pair j: 3x3 VALID conv as 36 shifted bf16
           matmuls -> psum; ct = psum*g + bias (DVE); H-up via Ulo/Uhi
           [128x128] matmuls (junk rows killed by zero filter rows);
           u1[w] = [128 hu, 512 co] bf16 -> DRAM (contiguous).
  phase 2: corner turn #1: chunked strided reads u1 -> ubig[w-part,
           hu x co] (hu<64 at partitions 0.., hu>=64 at 64..);
           per (m=co-half, hu): W-up matmul (K=62) -> Prelu (alternating
           ACT/DVE, sqrt2 folded into W-down matrix) -> W-down -> slab
           -> d2[hu] DRAM (corner turn #2).
  phase 3: chunked strided reads d2 -> et[hu-part, wd x co]; H-down
           matmuls -> out stores. m=0 H-down interleaved into m=1's
           W-stage to hide the turn-2 read latency.

Corner turns bounce through DRAM because SBUF->SBUF scatters into few
partitions run at ~20 GB/s (measured), while DRAM strided reads with
>=512B runs sustain ~190 GB/s and contiguous writes ~390 GB/s.
"""
import math
import os
import numpy as np
from contextlib import ExitStack

import concourse.bass as bass
import concourse.bacc as bacc
import concourse.tile as tile
from concourse import mybir
from concourse.bass_utils import run_bass_kernel_spmd

F32 = mybir.dt.float32
BF16 = mybir.dt.bfloat16
AF = mybir.ActivationFunctionType
ALU = mybir.AluOpType

B, CI, CO, H, W = 8, 512, 512, 64, 64
KS, TAPS, UP = 3, 12, 2
HO = H - 2                      # 62 valid conv outputs per axis
STYLE = 512
XPAD = 66 * 64                  # x tile free size: 64 cols x 66 rows
NB = HO // 2                    # 31 w-column-pair blocks
K9 = KS * KS
LIN_SCALE = 1.0 / math.sqrt(STYLE)
WSCALE = 1.0 / math.sqrt(CI * KS * KS)
EPS = 1e-8
SQRT2 = math.sqrt(2.0)
CQ = 256                        # co half for phases 2/3

_CACHE: dict = {}


def _build_nc(reps=1):
    nc = bacc.Bacc()

    xt_d = nc.declare_dram_parameter("xt", [4, 128, XPAD], BF16, isOutput=False)
    stylec_d = nc.declare_dram_parameter("stylec", [4, 128, 1], F32, isOutput=False)
    modw_d = nc.declare_dram_parameter("modw", [4, 128, 512], F32, isOutput=False)
    modb_d = nc.declare_dram_parameter("modb", [128, 4], F32, isOutput=False)
    qh_d = nc.declare_dram_parameter("qh", [4, 128, CO], F32, isOutput=False)
    cw_d = nc.declare_dram_parameter("cw", [4, 128, K9 * CO], BF16, isOutput=False)
    actb_d = nc.declare_dram_parameter("actb", [1, CO], F32, isOutput=False)
    ulo_d = nc.declare_dram_parameter("ulo", [128, 128], BF16, isOutput=False)
    uhi_d = nc.declare_dram_parameter("uhi", [128, 128], BF16, isOutput=False)
    uw_d = nc.declare_dram_parameter("uw", [128, 128], BF16, isOutput=False)
    dw_d = nc.declare_dram_parameter("dw", [128, 64], BF16, isOutput=False)
    dh_d = nc.declare_dram_parameter("dh", [128, 64], BF16, isOutput=False)
    out_d = nc.declare_dram_parameter("o", [64, 64, CO], F32, isOutput=True)

    u1_d = nc.dram_tensor("u1scratch", [HO, 128, CO], BF16)   # [w, hu, co]
    d2_d = nc.dram_tensor("d2scratch", [2, 64, 128, CQ], BF16)  # [m, wd, hu, co]

    with ExitStack() as ctx:
        tc = ctx.enter_context(tile.TileContext(nc))
        if reps > 1:
            ctx.enter_context(tc.For_i(0, reps, 1))
        pp = ctx.enter_context(tc.tile_pool(name="persist", bufs=1))

        ulo_t = pp.tile([128, 128], BF16)
        uhi_t = pp.tile([128, 128], BF16)
        uw_t = pp.tile([128, 128], BF16)
        dw_t = pp.tile([128, 64], BF16)
        dh_t = pp.tile([128, 64], BF16)
        g_b = pp.tile([128, CO], F32)
        bias_b = pp.tile([128, CO], F32)
        s_col = pp.tile([128, 4], F32)
        ubig = pp.tile([128, 64 * CO], BF16)     # [w(+64 for hu>=64), hu%64 x co]

        with tc.tile_pool(name="xw", bufs=1) as xw, \
             tc.tile_pool(name="pre", bufs=1) as pre, \
             tc.tile_pool(name="preps", bufs=1, space="PSUM") as preps, \
             tc.tile_pool(name="cpool", bufs=3) as cpool, \
             tc.tile_pool(name="upool", bufs=3) as upool, \
             tc.tile_pool(name="ps1", bufs=2, space="PSUM") as ps1, \
             tc.tile_pool(name="ps2", bufs=2, space="PSUM") as ps2:

            xts = [xw.tile([128, XPAD], BF16, tag=f"x{t}", name=f"x{t}") for t in range(4)]
            cws = [xw.tile([128, K9 * CO], BF16, tag=f"w{t}", name=f"w{t}") for t in range(4)]
            modw_t = [pre.tile([128, 512], F32, tag=f"mw{t}", name=f"mw{t}") for t in range(4)]
            styl_t = [pre.tile([128, 1], F32, tag=f"st{t}", name=f"st{t}") for t in range(4)]
            qh_t = [pre.tile([128, CO], F32, tag=f"qh{t}", name=f"qh{t}") for t in range(4)]
            modb_t = pre.tile([128, 4], F32)
            actb_t = pre.tile([1, CO], F32)
            ones_row = pre.tile([1, 128], F32)
            s2_t = pre.tile([128, 4], F32)
            sd_row = pre.tile([1, CO], F32)
            grow = pre.tile([1, CO], F32)
            eps_t = pre.tile([1, 1], F32)

            # q1 (sync): small prologue tensors first, then conv weights.
            for t in range(4):
                nc.sync.dma_start(out=modw_t[t], in_=modw_d[t])
                nc.sync.dma_start(out=styl_t[t], in_=stylec_d[t])
            nc.sync.dma_start(out=modb_t, in_=modb_d[:, :])
            nc.sync.dma_start(out=ulo_t, in_=ulo_d[:, :])
            nc.sync.dma_start(out=uhi_t, in_=uhi_d[:, :])
            nc.sync.dma_start(out=uw_t, in_=uw_d[:, :])
            nc.sync.dma_start(out=dw_t, in_=dw_d[:, :])
            nc.sync.dma_start(out=dh_t, in_=dh_d[:, :])
            for t in range(4):
                nc.sync.dma_start(out=qh_t[t], in_=qh_d[t])
            nc.sync.dma_start(out=actb_t, in_=actb_d[:, :])
            for t in range(4):
                nc.sync.dma_start(out=cws[t], in_=cw_d[t])
            # q10 (scalar): x tiles.
            for t in range(4):
                nc.scalar.dma_start(out=xts[t], in_=xt_d[t])
            nc.vector.memset(ones_row, 1.0)

            # s = style @ (mod_w*lin_scale).T + mod_b   (fp32 matvec)
            for cib in range(4):
                ps_s = preps.tile([128, 1], F32, tag="s")
                for dt_ in range(4):
                    nc.tensor.matmul(ps_s,
                                     modw_t[dt_][:, cib * 128:(cib + 1) * 128],
                                     styl_t[dt_], start=(dt_ == 0), stop=(dt_ == 3))
                nc.vector.scalar_tensor_tensor(
                    out=s_col[:, cib:cib + 1], in0=ps_s, scalar=1.0,
                    in1=modb_t[:, cib:cib + 1], op0=ALU.mult, op1=ALU.add)

            # scale x by s (per-ci partition scalar), in place, in two
            # w-halves so early conv blocks start sooner
            for t in range(4):
                nc.vector.tensor_scalar_mul(
                    xts[t][:, 0:2112], xts[t][:, 0:2112], s_col[:, t:t + 1])
            for t in range(4):
                nc.vector.tensor_scalar_mul(
                    xts[t][:, 2112:XPAD], xts[t][:, 2112:XPAD], s_col[:, t:t + 1])

            # g[co] = 1/sqrt(c2*A + c2*EPS/wscale^2), A = sum_ci s^2 Q
            nc.scalar.activation(out=s2_t, in_=s_col, func=AF.Square)
            ps_a = preps.tile([1, CO], F32, tag="A")
            for t in range(4):
                nc.tensor.matmul(ps_a, s2_t[:, t:t + 1], qh_t[t],
                                 start=(t == 0), stop=(t == 3))
            c2 = (1.0 + EPS) ** 2
            nc.vector.memset(eps_t, EPS * c2 / (WSCALE * WSCALE))
            nc.scalar.activation(out=sd_row, in_=ps_a, func=AF.Sqrt,
                                 scale=c2, bias=eps_t)
            nc.vector.reciprocal(grow, sd_row)

            # broadcast g and act_b across partitions via rank-1 matmuls
            ps_b = preps.tile([128, CO], F32, tag="bc")
            nc.tensor.matmul(ps_b, ones_row, grow, start=True, stop=True)
            nc.vector.tensor_copy(out=g_b, in_=ps_b)
            ps_b2 = preps.tile([128, CO], F32, tag="bc")
            nc.tensor.matmul(ps_b2, ones_row, actb_t, start=True, stop=True)
            nc.vector.tensor_copy(out=bias_b, in_=ps_b2)

            # ---- phase 1: conv + H-up, u1 streamed to DRAM per w column ----
            # x is [ci, w, h] column-major: lhsT for (j, kh, kw) is one
            # contiguous 128-run covering w-columns {2j+kw, 2j+kw+1}.  Bleed
            # rows land on junk partitions 62/63/126/127, killed by the zero
            # rows of Ulo/Uhi.
            cts = [None] * NB

            def hup(j):
                for dlt in range(2):
                    pu = ps2.tile([128, CO], F32, tag="uh")
                    um = ulo_t if dlt == 0 else uhi_t
                    nc.tensor.matmul(pu, um, cts[j], start=True, stop=True)
                    ut = upool.tile([128, CO], BF16, tag="u1")
                    nc.scalar.copy(out=ut, in_=pu)
                    nc.scalar.dma_start(out=u1_d[2 * j + dlt], in_=ut)

            def t1_read(gi, b0, b1, w0, w1):
                # ubig[w-part + 64*(hu>=64), hu%64 x co] <- u1[w0:w1, b0:b1, :]
                p0 = (0 if b0 < 64 else 64) + w0
                f0 = b0 % 64
                n = b1 - b0
                dst = ubig[p0:p0 + (w1 - w0), f0 * CO:(f0 + n) * CO]
                dst = dst.rearrange("p (a c) -> p a c", a=n)
                eng = nc.sync if gi % 2 == 0 else nc.scalar
                with nc.allow_non_contiguous_dma(reason="corner turn 1"):
                    eng.dma_start(out=dst, in_=u1_d[w0:w1, b0:b1, :])

            for j in range(NB):
                pc = ps1.tile([128, CO], F32, tag="conv")
                n_mm = 0
                for t in range(4):
                    for kh in range(KS):
                        for kw in range(KS):
                            rhs = cws[t][:, (kh * 3 + kw) * CO:
                                         (kh * 3 + kw + 1) * CO]
                            base = (2 * j + kw) * 64 + kh
                            lhs = xts[t][:, base:base + 128]
                            nc.tensor.matmul(pc, lhs, rhs,
                                             start=(n_mm == 0),
                                             stop=(n_mm == 35))
                            n_mm += 1
                ct = cpool.tile([128, CO], BF16, tag="c")
                nc.vector.tensor_mul(ct, pc, g_b)
                nc.vector.tensor_add(ct, ct, bias_b)
                cts[j] = ct
                if j >= 1:
                    hup(j - 1)
                if j == 24:
                    # prefetch 77% of corner turn 1 (w < 48 needs only
                    # u1 writes from j <= 23) while conv still runs
                    for gi in range(4):
                        t1_read(gi, 32 * gi, 32 * gi + 32, 0, 48)
            hup(NB - 1)

        # corner turn 1 leftovers (w 48..61), small hu chunks so W-up
        # starts right after the last u1 write
        for gi in range(8):
            t1_read(gi, 16 * gi, 16 * gi + 16, 48, HO)

        # ---- phases 2+3 per co-half m, processed in hu-PAIRS (strided
        # matmul rhs -> N=512 instructions): W-up -> Prelu (Act) -> W-down
        # -> slab8 (DVE) -> d2[m, wd, hu, co] (2KB runs, issues on SP).
        # H-down per 4-wd quad (copy alternates Act/DVE); m=0's H-down is
        # interleaved into m=1's W-stage to hide the turn-2 reads.  Engine
        # budget per pair @2.4GHz: PE 426ns | Act ~prelu 377 | DVE ~slab
        # 320 | SP issues.  Deep buffering (psw 4, a_t 6) keeps the PE fed
        # so it can ramp to max p-state.
        with tc.tile_pool(name="apool", bufs=6) as apool, \
             tc.tile_pool(name="spool", bufs=3) as spool, \
             tc.tile_pool(name="epool", bufs=4) as epool, \
             tc.tile_pool(name="opool", bufs=4) as opool, \
             tc.tile_pool(name="psw", bufs=4, space="PSUM") as psw, \
             tc.tile_pool(name="psd", bufs=2, space="PSUM") as psd, \
             tc.tile_pool(name="psh", bufs=1, space="PSUM") as psh:

            ets = {}

            def t2_reads(m, queues):
                with nc.allow_non_contiguous_dma(reason="corner turn 2"):
                    for c in range(8):
                        et = epool.tile([128, 8 * CQ], BF16, tag="et")
                        src = d2_d[m, 8 * c:8 * c + 8, :, :]
                        queues[c % len(queues)].dma_start(
                            out=et.rearrange("p (a c) -> p a c", a=8),
                            in_=src.transpose([1, 0, 2]))
                        ets[(m, c)] = et

            def hdown(m, wq):
                # one quad = 4 wd columns: 2 matmuls -> 1 copy -> 1 store
                wd = 4 * wq
                ph = psh.tile([64, 4 * CQ], F32, tag="ph")
                et = ets[(m, wd // 8)]
                for i in range(2):
                    rhs = et[:, (wd % 8 + 2 * i) * CQ:(wd % 8 + 2 * i + 2) * CQ]
                    nc.tensor.matmul(ph[:, 2 * i * CQ:(2 * i + 2) * CQ],
                                     dh_t, rhs, start=True, stop=True)
                st = opool.tile([64, 4 * CQ], F32, tag="st")
                if wq % 2 == 0:
                    nc.vector.tensor_copy(out=st, in_=ph)
                else:
                    nc.scalar.copy(out=st, in_=ph)
                with nc.allow_non_contiguous_dma(reason="out store"):
                    nc.sync.dma_start(
                        out=out_d[:, wd:wd + 4, m * CQ:(m + 1) * CQ],
                        in_=st.rearrange("p (a c) -> p a c", a=4))

            pws = [None] * 64
            slabs = [None] * 64

            def wup(m, pr):
                hu = 2 * pr
                k0 = 0 if hu < 64 else 64
                pw = psw.tile([128, 2 * CQ], F32, tag="pw")
                rhs = bass.AP(tensor=ubig.tensor,
                              offset=ubig.offset + k0 * ubig.ap[0][0]
                              + (hu % 64) * CO + m * CQ,
                              ap=[[ubig.ap[0][0], HO], [CO, 2], [1, CQ]])
                nc.tensor.matmul(pw, uw_t[k0:k0 + HO, :], rhs,
                                 start=True, stop=True)
                a_t = apool.tile([128, 2 * CQ], BF16, tag="a")
                if pr % 5 != 4:
                    nc.scalar.activation(out=a_t, in_=pw, func=AF.Prelu,
                                         alpha=0.2)
                else:
                    t0 = spool.tile([128, 2 * CQ], F32, tag="t0")
                    nc.vector.tensor_scalar(
                        out=t0, in0=pw, scalar1=0.0, scalar2=0.8,
                        op0=ALU.max, op1=ALU.mult)
                    nc.vector.scalar_tensor_tensor(
                        out=a_t, in0=pw, scalar=0.2, in1=t0,
                        op0=ALU.mult, op1=ALU.add)
                pws[pr] = a_t

            def wdown(m, pr):
                hu = 2 * pr
                pd = psd.tile([64, 2 * CQ], F32, tag="pd")
                nc.tensor.matmul(pd, dw_t, pws[pr], start=True, stop=True)
                if pr % 4 == 0:
                    slab = spool.tile([64, 8 * CQ], BF16, tag="sl")
                    slabs[pr] = slab
                else:
                    slab = slabs[pr - pr % 4]
                half = slab[:, (pr % 4) * 2 * CQ:(pr % 4 + 1) * 2 * CQ]
                if pr % 5 != 4:
                    nc.vector.tensor_copy(out=half, in_=pd)
                else:
                    nc.scalar.copy(out=half, in_=pd)
                if pr % 4 == 3:
                    with nc.allow_non_contiguous_dma(reason="d2 store"):
                        nc.sync.dma_start(
                            out=d2_d[m, :, hu - 6:hu + 2, :],
                            in_=slab.rearrange("p (a c) -> p a c", a=8))

            # W-stage m=0 (software-pipelined by 3 pairs)
            for pr in range(64):
                wup(0, pr)
                if pr >= 4:
                    wdown(0, pr - 4)
            for pr in range(60, 64):
                wdown(0, pr)

            # T2 reads for m=0 queue up behind the last d2 write
            t2_reads(0, [nc.sync])

            # W-stage m=1 with m=0's H-down interleaved
            hd_done = 0
            for pr in range(64):
                wup(1, pr)
                if pr >= 4:
                    wdown(1, pr - 4)
                want = 0 if pr < 8 else min(16, 1 + (pr - 8) * 15 // 55)
                while hd_done < want:
                    hdown(0, hd_done)
                    hd_done += 1
            for pr in range(60, 64):
                wdown(1, pr)
            while hd_done < 16:
                hdown(0, hd_done)
                hd_done += 1

            t2_reads(1, [nc.sync, nc.scalar])
            for wq in range(16):
                hdown(1, wq)

    nc.compile()
    return nc


def _host_prep(x, style, mod_w, mod_b, conv_w, act_b, up_filter, down_filter):
    import ml_dtypes
    bf16 = ml_dtypes.bfloat16
    x = np.asarray(x, np.float32)
    style = np.asarray(style, np.float32)
    mod_w = np.asarray(mod_w, np.float32)
    mod_b = np.asarray(mod_b, np.float32)
    conv_w = np.asarray(conv_w, np.float32)
    act_b = np.asarray(act_b, np.float32)
    up_filter = np.asarray(up_filter, np.float64)
    down_filter = np.asarray(down_filter, np.float64)

    # FIR matrices (see upfirdn2d in the reference):
    #   up:   y[o] = sum_i fu[o + 3 - 2i] x[i],   fu = up_filter * 2
    #   down: y[o] = sum_u df[2o + 6 - u] x[u]
    fu = up_filter * UP
    U = np.zeros((HO, 2 * H), np.float32)
    for i in range(HO):
        for o in range(2 * H):
            t = o + 3 - 2 * i
            if 0 <= t < TAPS:
                U[i, o] = fu[t]
    D = np.zeros((2 * H, H), np.float32)
    for u in range(2 * H):
        for o in range(H):
            t = 2 * o + 6 - u
            if 0 <= t < TAPS:
                D[u, o] = down_filter[t]
    ulo = np.zeros((128, 128), np.float32)
    uhi = np.zeros((128, 128), np.float32)
    uw = np.zeros((128, 128), np.float32)
    ulo[0:HO, :] = U
    uhi[64:64 + HO, :] = U
    uw[0:HO, :] = U
    uw[64:64 + HO, :] = U
    dw = SQRT2 * D          # sqrt2 lrelu gain folded into W-down
    dh = D

    modw_host = np.ascontiguousarray(
        (mod_w * LIN_SCALE).T.reshape(4, 128, 512), np.float32)
    modb_host = np.ascontiguousarray(mod_b.reshape(4, 128).T, np.float32)
    qh_host = np.ascontiguousarray(
        (conv_w.astype(np.float64) ** 2).sum(axis=(2, 3)).T
        .reshape(4, 128, CO), np.float32)
    cw_host = np.ascontiguousarray(
        conv_w.transpose(1, 2, 3, 0).reshape(4, 128, K9 * CO)).astype(bf16)
    actb_host = np.ascontiguousarray(act_b.reshape(1, CO), np.float32)

    # column-major x: [ci, w, h], flat + 128 zero pad at the end
    xp = np.zeros((B, 4, 128, XPAD), bf16)
    xcm = x.reshape(B, 4, 128, 64, 64).transpose(0, 1, 2, 4, 3)  # [.., w, h]
    xp[:, :, :, 0:64 * 64] = xcm.reshape(B, 4, 128, 64 * 64).astype(bf16)
    stylec = np.ascontiguousarray(style.reshape(B, 4, 128, 1), np.float32)

    shared = {
        "modw": modw_host, "modb": modb_host, "qh": qh_host, "cw": cw_host,
        "actb": actb_host, "ulo": ulo.astype(bf16), "uhi": uhi.astype(bf16),
        "uw": uw.astype(bf16), "dw": dw.astype(bf16), "dh": dh.astype(bf16),
    }
    in_maps = []
    for b in range(B):
        im = dict(shared)
        im["xt"] = np.ascontiguousarray(xp[b])
        im["stylec"] = stylec[b]
        in_maps.append(im)
    return in_maps


def kernel(**inputs):
    _install_neff_cache()
    if "nc" not in _CACHE:
        _CACHE["nc"] = _build_nc()
    nc = _CACHE["nc"]
    in_maps = _host_prep(**inputs)
    trace = os.environ.get("AFC_TRACE", "0") == "1"
    res = run_bass_kernel_spmd(nc, in_maps, list(range(B)), trace=trace)
    _CACHE["last_result"] = res
    out = np.stack([r["o"].transpose(2, 0, 1) for r in res.results])
    return np.ascontiguousarray(out, np.float32)


def _install_neff_cache():
    """Disk-cache walrus compiles by BIR hash (compile is ~10 min)."""
    import hashlib
    import shutil as _sh
    from concourse import bass_utils as _bu
    from concourse import bass2jax as _bj
    if getattr(_bu, "_afc_cache_installed", False):
        return
    orig = _bu.compile_bir_kernel
    cache_dir = "/tmp/afc_neff_cache"
    os.makedirs(cache_dir, exist_ok=True)

    def cached(bir_json, tmpdir, neff_name="file.neff"):
        data = bir_json if isinstance(bir_json, bytes) else bir_json.encode()
        h = hashlib.sha256(data).hexdigest()[:24]
        cpath = os.path.join(cache_dir, h + ".neff")
        dst = os.path.join(tmpdir, neff_name)
        if os.path.exists(cpath):
            _sh.copy(cpath, dst)
            return dst
        p = orig(bir_json, tmpdir, neff_name)
        try:
            _sh.copy(p, cpath)
        except OSError:
            pass
        return p

    _bu.compile_bir_kernel = cached
    _bj.compile_bir_kernel = cached
    _bu._afc_cache_installed = True


def _make_runner(nc, in_maps, k=1):
    """Build a reusable jitted shard_map callable over 8 cores with
    device-resident inputs (mirrors bass2jax.run_bass_via_pjrt). With k>1
    the NEFF executes k times per dispatch, chained through the donated
    output operands so XLA cannot CSE or parallelize the calls."""
    import jax
    from jax.experimental.shard_map import shard_map
    from jax.sharding import Mesh, NamedSharding, PartitionSpec
    from concourse import bass2jax

    bass2jax.install_neuronx_cc_hook()
    partition_name = nc.partition_id_tensor.name if nc.partition_id_tensor else None
    in_names, out_names, out_avals, zero_outs = [], [], [], []
    for alloc in nc.m.functions[0].allocations:
        if not isinstance(alloc, mybir.MemoryLocationSet):
            continue
        name = alloc.memorylocations[0].name
        if alloc.kind == "ExternalInput":
            if name != partition_name:
                in_names.append(name)
        elif alloc.kind == "ExternalOutput":
            out_names.append(name)
            shape = tuple(alloc.tensor_shape)
            dtype = mybir.dt.np(alloc.dtype)
            out_avals.append(jax.core.ShapedArray(shape, dtype))
            zero_outs.append(np.zeros(shape, dtype))
    n_params = len(in_names)
    all_names = list(in_names) + out_names
    if partition_name is not None:
        all_names.append(partition_name)

    def _body(*args):
        ins = list(args[:n_params])
        outs = list(args[n_params:])
        for _ in range(k):
            operands = ins + outs
            if partition_name is not None:
                operands.append(bass2jax.partition_id_tensor())
            outs = list(bass2jax._bass_exec_p.bind(
                *operands, out_avals=tuple(out_avals),
                in_names=tuple(all_names), out_names=tuple(out_names),
                lowering_input_output_aliases=(), sim_require_finite=True,
                sim_require_nnan=True, nc=nc))
        return tuple(outs)

    n = len(in_maps)
    devices = jax.devices()[:n]
    mesh = Mesh(np.asarray(devices), ("core",))
    nin = n_params + len(out_names)
    f = jax.jit(shard_map(_body, mesh=mesh,
                          in_specs=(PartitionSpec("core"),) * nin,
                          out_specs=(PartitionSpec("core"),) * len(out_names),
                          check_rep=False), keep_unused=True)
    sh = NamedSharding(mesh, PartitionSpec("core"))
    args = [jax.device_put(
        np.concatenate([np.asarray(m[nm]) for m in in_maps], axis=0), sh)
        for nm in in_names]
    args += [jax.device_put(
        np.zeros((n * z.shape[0], *z.shape[1:]), z.dtype), sh)
        for z in zero_outs]
    return f, args


def _time_runner(f, args, iters):
    import time as _time
    for _ in range(2):
        jax.block_until_ready(f(*args))
    best = float("inf")
    for _ in range(iters):
        t0 = _time.perf_counter()
        jax.block_until_ready(f(*args))
        best = min(best, _time.perf_counter() - t0)
    return best


def time_kernel(iters=6, k=24, **inputs):
    """Per-execution time via an in-kernel For_i repeat loop: the whole
    pipeline runs 1x and (1+k)x per dispatch; the difference isolates
    device time from the ~80 ms axon dispatch overhead. Returns ns."""
    global jax
    import jax
    _install_neff_cache()
    if "nc" not in _CACHE:
        _CACHE["nc"] = _build_nc()
    in_maps = _host_prep(**inputs)
    nck = _build_nc(reps=1 + k)
    f1, args = _make_runner(_CACHE["nc"], in_maps)
    fk, _ = _make_runner(nck, in_maps)
    t1 = _time_runner(f1, args, iters)
    tk = _time_runner(fk, args, iters)
    print(f"wall 1x: {t1*1e3:.2f} ms, {1+k}x: {tk*1e3:.2f} ms")
    return (tk - t1) / k * 1e9
